# revision 1
# baseline (speedup 1.0000x reference)
"""Classical self-attention on 8 trn2 NeuronCores.

N=16384 tokens, d=64, fp32. Sequence-parallel over Q: core c handles rows
[c*2048, (c+1)*2048). K/V computed redundantly on every core from the full x.

Math (reference):
  q = (x @ rot.T) @ Wq.T + bq = x @ (Wq@rot).T + bq
  k = x @ (Wk@ent).T + bk ;  v = x @ Wv.T + bv
  y = softmax(q @ k.T / 8) @ v

v2 design (v1's two-pass structure, rebalanced across engines):
  pass A (row max): fp8e4 DoubleRow matmuls (0.5 cyc/row) over d-split
    shadows kB8/qB8 [32, 2, *] (d = slot*32 + partition); per-tile row
    maxes via either
      - DVE: reduce_max over each psA pair [128,1024] -> mms -> mt, or
      - ACT+Pool (group-0 tiles only, when ACT has no exp work yet):
        ACT copies psA pairs to bf16 SBUF, Pool runs a running
        elementwise max, DVE does one final reduce.
    (Pool/DMA cannot read PSUM; TensorTensor allows only one PSUM
    operand - all hardware-verified, so these are the only legal splits.)
  pass B: sT = kB.T @ qB in fp32r (1 cyc/row), -m folded via ones row.
  exp on ACT [128,1024] per psB pair; PV in fp32r accumulating psPV.
  Schedule: per-cc setup emits kB/kB8 then A(g0) then V quads; phases
  p=1..4 run B/PV/exp(g_{p-1}) with A(g_p) pairs spread between units.
"""

import sys

sys.path.insert(0, "/opt/trn_rl_repo")

from contextlib import ExitStack

import numpy as np

import concourse.bass as bass
import concourse.mybir as mybir
import concourse.tile as tile
from concourse import bacc
from concourse.bass import ds, ts
from concourse.bass_utils import run_bass_kernel_spmd

N_CORES = 8
N = 16384
D = 64
QR = N // N_CORES          # 2048 q rows per core
N_QTILE = QR // 128        # 16 q tiles per core
N_GROUP = 4                # groups of 4 q-tiles (512 q cols)
TILES_PER_GROUP = N_QTILE // N_GROUP
N_KV_BLK = N // 128        # 128 kv blocks
A_CHUNK = 512              # pass-A kv chunk (psum half)
N_A_CHUNKS = N // A_CHUNK  # 32 chunks per q tile
N_PAIRS = N_A_CHUNKS // 2  # 16 psA pairs per q tile
F32 = mybir.dt.float32
BF16 = mybir.dt.bfloat16
F8 = mybir.dt.float8e4
R32 = mybir.dt.float32r
DR = mybir.MatmulPerfMode.DoubleRow

_CACHED = {}

# q tiles whose row-max reduction goes via ACT-copy + Pool running max.
# DEAD: walrus codegen rejects TensorTensor on Pool (engine check) - Pool
# only lowers tensor_copy / partition_all_reduce / memset / DMA. Keep ()
POOL_TILES = ()


def build_kernel():
    nc = bacc.Bacc("TRN2", target_bir_lowering=False, debug=False,
                   num_devices=N_CORES)

    x_d = nc.dram_tensor("x", [N, D], F32, kind="ExternalInput")
    xq_d = nc.dram_tensor("xq", [QR, D], F32, kind="ExternalInput")
    wq_d = nc.dram_tensor("wq", [D + 1, D], F32, kind="ExternalInput")
    wk_d = nc.dram_tensor("wk", [D + 1, D], F32, kind="ExternalInput")
    wv_d = nc.dram_tensor("wv", [D + 1, D], F32, kind="ExternalInput")
    id_d = nc.dram_tensor("ident", [128, 128], F32, kind="ExternalInput")
    y_d = nc.dram_tensor("y", [QR, D], F32, kind="ExternalOutput")

    with tile.TileContext(nc) as tc, ExitStack() as ctx:
        sb = ctx.enter_context(tc.tile_pool(name="sb", bufs=1))
        xtp = ctx.enter_context(tc.tile_pool(name="xtp", bufs=2))
        expp = ctx.enter_context(tc.tile_pool(name="expp", bufs=2))
        smp = ctx.enter_context(tc.tile_pool(name="smp", bufs=4))
        stgp = ctx.enter_context(tc.tile_pool(name="stgp", bufs=2))
        psA_p = ctx.enter_context(tc.tile_pool(name="psA", bufs=1, space="PSUM"))
        psB_p = ctx.enter_context(tc.tile_pool(name="psB", bufs=1, space="PSUM"))
        psPV_p = ctx.enter_context(tc.tile_pool(name="psPV", bufs=1, space="PSUM"))
        psM_p = ctx.enter_context(tc.tile_pool(name="psM", bufs=1, space="PSUM"))

        # ---- persistent SBUF ----
        kB = sb.tile([D + 1, N], F32)
        qB = sb.tile([D + 1, QR], F32)
        kB8 = sb.tile([32, 2, N], F8)    # d-split fp8 shadows
        qB8 = sb.tile([32, 2, QR], F8)
        V = sb.tile([128, N_KV_BLK, D + 1], F32)
        OT = sb.tile([D + 1, QR], F32)
        y_sb = sb.tile([128, N_QTILE, D], F32)
        wq = sb.tile([D + 1, D], F32)
        wk = sb.tile([D + 1, D], F32)
        wv = sb.tile([D + 1, D], F32)
        ident = sb.tile([128, 128], F32)
        mms = sb.tile([128, N_QTILE, N_PAIRS], F32)
        Mrun = sb.tile([128, max(len(POOL_TILES), 1), 1024], BF16)

        def rB(ap):
            return ap.bitcast(R32)

        rPV = rB

        nc.gpsimd.dma_start(wq[:], wq_d[:])
        nc.gpsimd.dma_start(wk[:], wk_d[:])
        nc.gpsimd.dma_start(wv[:], wv_d[:])
        nc.gpsimd.dma_start(ident[:], id_d[:])

        nc.gpsimd.memset(kB[D : D + 1, :], 1.0)
        nc.gpsimd.memset(V[:, :, D : D + 1], 1.0)
        nc.gpsimd.memset(Mrun[:], -3.0e38)

        # ---- PSUM tiles ----
        psA = psA_p.tile([128, 2 * A_CHUNK], F32)      # 2 banks, 1 pair
        psB = psB_p.tile([128, 2048], F32)             # 4 banks
        psPV = psPV_p.tile([128, 512], F32)            # 1 bank

        def build_xT(xT, dram_ap, ntiles):
            """dram_ap: [ntiles*128, 64] -> xT[0:D, 0:ntiles*128] via PE."""
            for h in range(0, ntiles, 16):
                nh = min(16, ntiles - h)
                xn = xtp.tile([128, 16, D], F32, tag="xn")
                nc.sync.dma_start(
                    xn[:, 0:nh, :],
                    dram_ap[ds(h * 128, nh * 128), :].rearrange(
                        "(j p) d -> p j d", p=128))
                for j4 in range(nh // 4):
                    pm = psM_p.tile([D, 4, 128], F32, tag="psm")
                    for jj in range(4):
                        nc.tensor.transpose(pm[:, jj, :], xn[:, j4 * 4 + jj, :],
                                            ident[:])
                    nc.scalar.copy(xT[0:D, ds(h * 128 + j4 * 512, 512)], pm[:])

        # ---- pass-A plumbing ----
        pool_slot = {t: i for i, t in enumerate(POOL_TILES)}
        mt_tiles = {}

        def emit_A_pair(t, p):
            """A-matmuls for chunks (2p, 2p+1) of tile t + max consumption."""
            for h in range(2):
                c = 2 * p + h
                nc.tensor.matmul(
                    psA[:, ds(h * A_CHUNK, A_CHUNK)],
                    qB8[:, :, ts(t, 128)],
                    kB8[:, :, ds(c * A_CHUNK, A_CHUNK)],
                    start=True, stop=True, perf_mode=DR)
            if t in pool_slot:
                stg = stgp.tile([128, 1024], BF16, tag="stg")
                nc.scalar.copy(stg[:], psA[:])
                s = pool_slot[t]
                nc.gpsimd.tensor_tensor(Mrun[:, s, :], stg[:], Mrun[:, s, :],
                                        op=mybir.AluOpType.max)
            else:
                nc.vector.reduce_max(mms[:, t, p : p + 1], psA[:],
                                     axis=mybir.AxisListType.X)

        def emit_A_final(t):
            mt = smp.tile([128, 1], F32, tag="mt")
            if t in pool_slot:
                nc.vector.reduce_max(mt[:], Mrun[:, pool_slot[t], :],
                                     axis=mybir.AxisListType.X)
            else:
                nc.vector.reduce_max(mt[:], mms[:, t, :],
                                     axis=mybir.AxisListType.X)
            mt_tiles[t] = mt

        def emit_mfin(t):
            """qB row 64 for tile t <- -mt via PE transpose (deferred)."""
            mt = mt_tiles.pop(t)
            pneg = psM_p.tile([1, 128], F32, tag="psm")
            nc.tensor.matmul(pneg[:], mt[:], ident[:], start=True, stop=True)
            nc.scalar.mul(rB(qB[D : D + 1, ts(t, 128)]), pneg[:], -1.0)

        # ---- setup: xq -> qB + qB8 ----
        xqT = xtp.tile([D + 1, QR], F32, tag="xt")
        build_xT(xqT, xq_d[:], N_QTILE)
        nc.gpsimd.memset(xqT[D : D + 1, :], 1.0)
        for j in range(QR // 512):
            pm = psM_p.tile([128, 512], F32, tag="psm")
            nc.tensor.matmul(pm[0:D, :], wq[:], xqT[:, ts(j, 512)],
                             start=True, stop=True)
            nc.scalar.copy(rB(qB[0:D, ts(j, 512)]), pm[0:D, :])
        nc.scalar.copy(qB8[:, 0, :], qB[0:32, :])
        nc.vector.tensor_copy(qB8[:, 1, :], qB[32:64, :])

        # ---- setup per cc (2048 tokens): kB, kB8, A(g0), V quads ----
        for cc in range(8):
            xT = xtp.tile([D + 1, 2048], F32, tag="xt")
            build_xT(xT, x_d[ds(cc * 2048, 2048), :], 16)
            nc.gpsimd.memset(xT[D : D + 1, :], 1.0)
            for j in range(4):
                pm = psM_p.tile([128, 512], F32, tag="psm")
                nc.tensor.matmul(pm[0:D, :], wk[:], xT[:, ts(j, 512)],
                                 start=True, stop=True)
                sl = ds(cc * 2048 + j * 512, 512)
                nc.scalar.copy(rB(kB[0:D, sl]), pm[0:D, :])
            ccsl = ds(cc * 2048, 2048)
            nc.scalar.copy(kB8[:, 0, ccsl], kB[0:32, ccsl])
            nc.vector.tensor_copy(kB8[:, 1, ccsl], kB[32:64, ccsl])
            # A(g0) over this cc's 2 pairs, interleaved with V quads so the
            # in-order PE queue always has work while DVE/ACT drain psA
            av_units = [(t, p)
                        for t in range(TILES_PER_GROUP)
                        for p in range(cc * 2, cc * 2 + 2)]
            for j, (t, p) in enumerate(av_units):
                emit_A_pair(t, p)
                if j % 2 == 1:
                    jj = j // 2
                    pm = psM_p.tile([128, 4, D], F32, tag="psm")
                    for q4 in range(4):
                        nc.tensor.matmul(
                            pm[:, q4, :],
                            xT[:, ts(jj * 4 + q4, 128)], wv[:],
                            start=True, stop=True)
                    b0 = cc * 16 + jj * 4
                    nc.scalar.copy(rPV(V[:, ds(b0, 4), 0:D]), pm[:])
        for t in range(TILES_PER_GROUP):
            emit_A_final(t)

        # ---- phases ----
        pv_pending = []

        def emit_PV_pending():
            while pv_pending:
                bb, ex_ap = pv_pending.pop(0)
                nc.tensor.matmul(psPV[0 : D + 1, :], rPV(V[:, bb, :]),
                                 rPV(ex_ap),
                                 start=(bb == 0), stop=(bb == N_KV_BLK - 1),
                                 skip_group_check=True)

        def emit_B(g, b):
            off = (b % 4) * 512
            nc.tensor.matmul(psB[:, ds(off, 512)], rB(kB[:, ts(b, 128)]),
                             rB(qB[:, ds(g * 512, 512)]), start=True, stop=True)
            if b % 2 == 1:
                emit_PV_pending()
                hoff = ((b // 2) % 2) * 1024
                ex = expp.tile([128, 1024], F32, tag="ex")
                nc.scalar.activation(rPV(ex[:]), psB[:, ds(hoff, 1024)],
                                     mybir.ActivationFunctionType.Exp)
                pv_pending.append((b - 1, ex[:, ds(0, 512)]))
                pv_pending.append((b, ex[:, ds(512, 512)]))

        # phase 0: just finalize m(g0)
        for t in range(TILES_PER_GROUP):
            emit_mfin(t)

        for phase in range(1, N_GROUP + 1):
            g = phase - 1          # group whose B/PV/exp runs this phase
            a_pairs = []
            if phase < N_GROUP:    # group whose pass A runs this phase
                for tt in range(TILES_PER_GROUP):
                    t = phase * TILES_PER_GROUP + tt
                    for p in range(N_PAIRS):
                        a_pairs.append((t, p))
            b_units = [(g, b) for b in range(N_KV_BLK)]

            nu = len(b_units)
            for u in range(nu):
                lo = (u * len(a_pairs)) // nu
                hi = ((u + 1) * len(a_pairs)) // nu
                for i in range(lo, hi):
                    emit_A_pair(*a_pairs[i])
                emit_B(*b_units[u])
            emit_PV_pending()
            nc.scalar.copy(OT[:, ds(g * 512, 512)], psPV[0 : D + 1, :])
            if phase < N_GROUP:
                for tt in range(TILES_PER_GROUP):
                    t = phase * TILES_PER_GROUP + tt
                    emit_A_final(t)
                    emit_mfin(t)

        # ---- final: transpose OT, normalize, store ----
        for t in range(N_QTILE):
            pO = psM_p.tile([128, D + 1], F32, tag="psm")
            nc.tensor.matmul(pO[:], OT[:, ts(t, 128)],
                             ident[0 : D + 1, 0 : D + 1],
                             start=True, stop=True)
            rz = smp.tile([128, 1], F32, tag="rz")
            nc.vector.reciprocal(rz[:], pO[:, D : D + 1])
            nc.vector.tensor_scalar_mul(y_sb[:, t, :], pO[:, 0:D], rz[:])
        nc.sync.dma_start(y_d.rearrange("(t p) d -> p t d", p=128), y_sb[:])

    nc.compile()
    return nc


def _prep_inputs(x, params, Wq, bq, Wk, bk, Wv, bv):
    x = np.ascontiguousarray(x, dtype=np.float32)
    params = np.asarray(params, dtype=np.float32)
    rot = params[:, :D]
    ent = params[:, D : 2 * D]
    scale = np.float32(1.0 / np.sqrt(D))
    wq_eff = (np.asarray(Wq, np.float32) @ rot)
    wk_eff = (np.asarray(Wk, np.float32) @ ent)
    wq = np.vstack([wq_eff.T, np.asarray(bq, np.float32)[None]]) * scale
    wk = np.vstack([wk_eff.T, np.asarray(bk, np.float32)[None]])
    wv = np.vstack([np.asarray(Wv, np.float32).T,
                    np.asarray(bv, np.float32)[None]])
    ident = np.eye(128, dtype=np.float32)
    return x, np.ascontiguousarray(wq), np.ascontiguousarray(wk), \
        np.ascontiguousarray(wv), ident


def kernel(x, params, Wq, bq, Wk, bk, Wv, bv, _trace=False):
    x, wq, wk, wv, ident = _prep_inputs(x, params, Wq, bq, Wk, bk, Wv, bv)
    if "nc" not in _CACHED:
        _CACHED["nc"] = build_kernel()
    nc = _CACHED["nc"]
    in_maps = []
    for c in range(N_CORES):
        in_maps.append({
            "x": x,
            "xq": np.ascontiguousarray(x[c * QR : (c + 1) * QR]),
            "wq": wq, "wk": wk, "wv": wv, "ident": ident,
        })
    res = run_bass_kernel_spmd(nc, in_maps, core_ids=list(range(N_CORES)),
                               trace=_trace)
    out = np.concatenate([res.results[c]["y"] for c in range(N_CORES)], axis=0)
    global _CACHED_RES
    _CACHED_RES = res
    return out

